# revision 33
# baseline (speedup 1.0000x reference)
"""Trainium2 Bass kernel for nn_CameraViewTransformerLSS (LSS camera->BEV transformer).

Pipeline (B=1, N=6 cams, D=48 depth bins, 64x176 feature map, C=80 ctx channels,
128x128 BEV grid, 128 output channels):

  1. lift:    feat[n,d,h,w,c] = depth_prob[n,d,h,w] * context[n,c,h,w]
  2. splat:   scatter-add feat into BEV bins by frustum geometry
  3. head:    1x1 conv (80->128) + BN + ReLU

Key structural fact: with this camera rig (rotations about z only), the BEV bin
of a frustum point depends only on (camera n, depth d, image column w) -- NOT on
the image row h.  So the h axis is contracted *before* any scatter:

  partial[(n,w,d), c] = sum_h depth[n,d,h,w] * ctx[n,c,h,w]     (a small matmul
  per camera-column "ray", K=h=64), reducing the scatter from 3.24M points to
  ~40K valid points.

Two launches, host shuffles (pure data movement) between them:

  L1 (ray-sharded lift): 132 rays/core; depth (48 cols) and ctx (80 cols) of a
      ray pair are packed side by side into one 128-col block.  Outputs packed
      into 96 PSUM partitions (2 rays x 48 d) -> part write is 96 x rays x d x c
      bf16 per core.
  host (free): BEV bins are re-numbered: nonzero-count bins only (zero bins get
      the constant ReLU(bias) on host -- weight-derived), sorted by point count
      and snake-dealt to cores; each core's bins form S slots of 128 columns.
      A point's row WITHIN a scatter tile equals its bin's column, so the
      scatter one-hot collapses to a shared IDENTITY matrix for most tiles
      ("identity tiles": tile k of a slot holds the k-th point of every bin
      with count > k).  Heavy-bin overflow points go to classic one-hot tiles.
  L2 (bin-sharded scatter + head): per 4-slot PSUM bank: accumulate identity /
      one-hot matmuls (lhsT = 128 points x 80c), then 1x1-conv matmul (80->128,
      BN scale folded into weight), BN-bias + ReLU out of PSUM, bf16 y write.

Bin indices are computed on host with jnp mirroring the reference op sequence
exactly (same backend => identical floor results).
"""

import functools
import math

import numpy as np

import concourse.bacc as bacc
import concourse.mybir as mybir
import concourse.tile as tile
from concourse.bass_utils import run_bass_kernel_spmd

# ---------------------------------------------------------------- constants
NCAM, DD, HF, WF, CC = 6, 48, 64, 176, 80
BH = BW = 128
OC = 128
STRIDE = 4.0
PC = (-50.0, -50.0, -5.0, 50.0, 50.0, 3.0)
Z_MIN, Z_MAX = 1.0, 60.0
BN_EPS = 1e-5

NCORES = 8
RAYS = NCAM * WF            # 1056
RPC = RAYS // NCORES        # 132 rays per core
GPC = RPC // 4              # 33 groups of 4 rays
F32 = mybir.dt.float32
BF16 = mybir.dt.bfloat16

PAIR = DD + CC              # 128: depth cols + ctx cols of one ray


def _np_bf16():
    import ml_dtypes

    return np.dtype(ml_dtypes.bfloat16)


# ------------------------------------------------------------- L1 flipped
def _l1_geom(dlen):
    """Derive per-ray input blocks, PSUM-bank windows and chunking from the
    uniform per-position valid-d lengths.  All data lives in partitions 0:64
    (one ray per column block: [dlen depth | 80 ctx]), so every matmul uses
    the same PE tile position (lhsT base 0, out base 0) -- mixing tile
    positions with overlapping output partitions faults the NEFF.  A ray's
    partial occupies a [CC, dlen] window of a 512-col PSUM bank (c on
    partitions, d on the free axis); the last window of each bank is widened
    to fill the bank so the bank copy reads only written PSUM."""
    RP = len(dlen)
    base = [0]
    for r in range(RP):
        base.append(base[-1] + dlen[r] + CC)
    win = []                                 # (bank, off, width) per position
    bank, off = 0, 0
    for r in range(RP):
        wd = dlen[r]
        if off + wd > 512:
            win[-1] = (win[-1][0], win[-1][1], 512 - win[-1][1])  # widen last
            bank += 1
            off = 0
        win.append((bank, off, wd))
        off += wd
    nbank = bank + 1
    ncols = (nbank - 1) * 512 + off          # written output cols
    return base, win, nbank, ncols


@functools.lru_cache(maxsize=2)
def _build_l1f(dlen):
    """Flipped lift: per ray one matmul, lhsT = ctx (64h x 80c), rhs = depth
    (64h x dlen d) -> out [80c, dlen] in a PSUM-bank window.  Output is then
    exactly the valid partials: part[80, nbank*512] bf16."""
    base, win, nbank, ncols = _l1_geom(dlen)
    RP = len(dlen)
    cols_in = base[-1]
    nc = bacc.Bacc("TRN2", target_bir_lowering=False, debug=False, num_devices=NCORES)
    dc_in = nc.dram_tensor("dc_in", [64, cols_in], BF16, kind="ExternalInput")
    part = nc.dram_tensor("part", [CC, ncols], BF16, kind="ExternalOutput")

    # input chunks at ray boundaries: small first chunk, then ~even fifths
    cb = [0, 4]
    for i in range(1, 6):
        cb.append(round(4 + (RP - 4) * i / 5))
    cb[-1] = RP

    with tile.TileContext(nc) as tc:
        with (
            tc.tile_pool(name="dc", bufs=4) as dc_pool,
            tc.tile_pool(name="stage", bufs=1) as stage_pool,
            tc.tile_pool(name="ps", bufs=3, space="PSUM") as ps_pool,
            tc.tile_pool(name="warm", bufs=1) as warm_pool,
        ):
            warm = warm_pool.tile([128, 2], BF16)
            nc.gpsimd.memset(warm[:, 0:1], 0)
            nc.scalar.activation(
                out=warm[:, 1:2], in_=warm[:, 0:1],
                func=mybir.ActivationFunctionType.Relu,
            )
            stage = stage_pool.tile([CC, ncols], BF16)
            pt = None
            cur_tile = -1
            out_lo = 0

            def drain(tl, last):
                nonlocal out_lo
                lo = tl * 1024
                hi = min((tl + 1) * 1024, ncols)
                if tl % 2 == 0:
                    nc.vector.tensor_copy(out=stage[:, lo:hi], in_=pt[0:CC, 0:hi - lo])
                else:
                    nc.scalar.copy(out=stage[:, lo:hi], in_=pt[0:CC, 0:hi - lo])
                # stream partials out; final chunk on HWDGE (no SWDGE
                # desc-gen in the tail), earlier chunks on the Pool queue
                if last or (tl % 2 == 1):
                    oeng = nc.sync if last else nc.gpsimd
                    oeng.dma_start(out=part[:, out_lo:hi], in_=stage[:, out_lo:hi])
                    out_lo = hi

            for ch in range(6):
                r0, r1 = cb[ch], cb[ch + 1]
                if r1 == r0:
                    continue
                clo, chi = base[r0], base[r1]
                dct = dc_pool.tile([64, chi - clo], BF16, tag="dc")
                nc.sync.dma_start(out=dct[:], in_=dc_in[:, clo:chi])
                for r in range(r0, r1):
                    bk, off, wd = win[r]
                    if bk // 2 != cur_tile:
                        if cur_tile >= 0:
                            drain(cur_tile, False)
                        cur_tile = bk // 2
                        pt = ps_pool.tile([128, 1024], F32, space="PSUM")
                    b = base[r] - clo
                    nc.tensor.matmul(
                        out=pt[0:CC, (bk % 2) * 512 + off:(bk % 2) * 512 + off + wd],
                        lhsT=dct[0:64, b + dlen[r]:b + dlen[r] + CC],
                        rhs=dct[0:64, b:b + wd],
                        start=True,
                        stop=True,
                    )
            drain(cur_tile, True)
    nc.compile()
    return nc


def _l1f_inputs(depth_prob, context, plan):
    dt = _np_bf16()
    dlen = plan["dlen"]
    base, win, nbank, ncols = _l1_geom(dlen)
    dp = depth_prob[0]                       # (N, D, H, W)
    cx = context[0]                          # (N, C, H, W)
    dlo, lens = plan["dlo"], plan["lens"]
    maps = []
    for c in range(NCORES):
        order = plan["ray_order"][c]
        dc = np.zeros((64, base[-1]), np.float32)
        for r, ray in enumerate(order):
            n, w = divmod(int(ray), WF)
            lo, ln = int(dlo[ray]), int(lens[ray])
            dc[:, base[r]:base[r] + ln] = dp[n, lo:lo + ln, :, w].T
            dc[:, base[r] + dlen[r]:base[r] + dlen[r] + CC] = cx[n, :, :, w].T
        maps.append({"dc_in": dc.astype(dt)})
    return maps


def _unpack_l1f(out_cores, plan):
    """part[80, nbank*512] per core -> part_all[RAYS*DD, CC] (valid rows)."""
    dlen = plan["dlen"]
    _, win, _, _ = _l1_geom(dlen)
    dlo, lens = plan["dlo"], plan["lens"]
    part_all = np.zeros((RAYS * DD, CC), np.float32)
    for c in range(NCORES):
        pc = np.asarray(out_cores[c], np.float32)      # (80, nbank*512)
        order = plan["ray_order"][c]
        for r, ray in enumerate(order):
            bk, off, _ = win[r]
            lo, ln = int(dlo[ray]), int(lens[ray])
            sl = pc[:, bk * 512 + off:bk * 512 + off + ln]
            part_all[ray * DD + lo:ray * DD + lo + ln] = sl.T
    return part_all


# ---------------------------------------------------------------- L1 builder
@functools.lru_cache(maxsize=2)
def _build_l1():
    nc = bacc.Bacc("TRN2", target_bir_lowering=False, debug=False, num_devices=NCORES)
    # per ray pair j: cols [j*128, j*128+48) = depth, [j*128+48, (j+1)*128) = ctx
    # partitions 0:64 = h of first ray, 64:128 = h of second ray
    dc_in = nc.dram_tensor("dc_in", [128, GPC * 2 * PAIR], BF16, kind="ExternalInput")
    part = nc.dram_tensor("part", [112, GPC * 2 * CC], BF16, kind="ExternalOutput")

    CHS = (2, 4, 9, 9, 9)        # groups per input chunk (small first chunk
    BK = 512                     # so compute starts early)
    OBND = (5, 11, 17, 23, 29, 32)       # output chunk ends (group index)

    with tile.TileContext(nc) as tc:
        with (
            tc.tile_pool(name="dc", bufs=4) as dc_pool,
            tc.tile_pool(name="stage", bufs=1) as stage_pool,
            tc.tile_pool(name="ps", bufs=2, space="PSUM") as ps_pool,
            tc.tile_pool(name="warm", bufs=1) as warm_pool,
        ):
            # prewarm the Activation function table (1.28us load) at t=0 so
            # it is off the critical path of the first real Act copy
            warm = warm_pool.tile([128, 2], BF16)
            nc.gpsimd.memset(warm[:, 0:1], 0)
            nc.scalar.activation(
                out=warm[:, 1:2], in_=warm[:, 0:1],
                func=mybir.ActivationFunctionType.Relu,
            )
            stage = stage_pool.tile([112, GPC * 160], BF16)
            g0 = 0
            olo = 0
            for ch, GCH in enumerate(CHS):
                dct = dc_pool.tile([128, GCH * 2 * PAIR], BF16, tag="dc")
                nc.sync.dma_start(
                    out=dct[:], in_=dc_in[:, g0 * 2 * PAIR:(g0 + GCH) * 2 * PAIR]
                )
                for gg in range(GCH):
                    g = g0 + gg
                    # 4-bank PSUM tile shared by 2 groups (4 ray pairs); one
                    # wide copy per 2 groups halves the copy init overhead.
                    if g % 2 == 0:
                        pt = ps_pool.tile([128, 4 * BK], F32, space="PSUM")
                    for pj in range(2):            # pair index within group
                        j = 2 * gg + pj
                        bank = 2 * (g % 2) + pj
                        dsl = slice(j * PAIR, j * PAIR + DD)
                        csl = slice(j * PAIR + DD, (j + 1) * PAIR)
                        # ray 4g+pj (PE rows 0:64) -> partitions 0:48.
                        # lhsT widened to 64 cols (16 ctx cols as junk
                        # weights) so PSUM rows 48:64 are initialized for the
                        # block copy; rows 48:64 are dropped at host unpack.
                        nc.tensor.matmul(
                            out=pt[0:64, bank * BK:bank * BK + CC],
                            lhsT=dct[0:64, slice(j * PAIR, j * PAIR + 64)],
                            rhs=dct[0:64, csl],
                            start=True,
                            stop=True,
                        )
                        # ray 4g+2+pj (PE rows 64:128) -> partitions 48:96
                        nc.tensor.matmul(
                            out=pt[64:64 + DD, bank * BK:bank * BK + CC],
                            lhsT=dct[64:128, dsl],
                            rhs=dct[64:128, csl],
                            start=True,
                            stop=True,
                        )
                    if g % 2 == 1 or g == GPC - 1:
                        nb = 2 * (g % 2) + 2        # banks filled in this tile
                        blo = (g // 2) * 2          # first group in the tile
                        src = pt[0:112, 0:nb * BK].rearrange(
                            "p (b x) -> p b x", b=nb
                        )[:, :, 0:CC]
                        dst = stage[:, blo * 160:(g + 1) * 160].rearrange(
                            "p (b x) -> p b x", b=nb
                        )
                        if (g // 2) % 2 == 0:
                            nc.scalar.copy(out=dst, in_=src)
                        else:
                            nc.vector.tensor_copy(out=dst, in_=src)
                    # stream partials out on the idle GPSIMD SWDGE queue
                    # (final chunk on HWDGE: no ~1us SWDGE desc-gen in tail)
                    if g in OBND:
                        lo = olo * 160
                        hi = (g + 1) * 160
                        olo = g + 1
                        oeng = nc.sync if g == GPC - 1 else nc.gpsimd
                        oeng.dma_start(out=part[:, lo:hi], in_=stage[:, lo:hi])
                g0 += GCH
    nc.compile()
    return nc


# Unpack map for L1 "part" output (112 partitions; rows 48:64 are junk):
#   parts 0:48   : [:, g, 0] = ray 4g+0, [:, g, 1] = ray 4g+1
#   parts 64:112 : [:, g, 0] = ray 4g+2, [:, g, 1] = ray 4g+3
def _unpack_l1(out_core):
    S = out_core.reshape(112, GPC, 2, CC)
    p = np.empty((RPC, DD, CC), out_core.dtype)
    p[0::4] = S[0:DD, :, 0].transpose(1, 0, 2)
    p[1::4] = S[0:DD, :, 1].transpose(1, 0, 2)
    p[2::4] = S[64:112, :, 0].transpose(1, 0, 2)
    p[3::4] = S[64:112, :, 1].transpose(1, 0, 2)
    return p


# ---------------------------------------------------------------- L2 builder
@functools.lru_cache(maxsize=8)
def _build_l2(bank_tiles, b1):
    """bank_tiles: per PSUM bank, a tuple of per-slot (n_ident, n_onehot)
    chains (uniform across cores).  Each slot is 128 bin-columns; up to 4
    slots per bank.  Scatter matmuls accumulate identity-rhs tiles (a point's
    row equals its bin's column) plus built one-hots for heavy-bin overflow
    into PSUM; per bank: copy to SBUF, 1x1-conv matmul (80->128, BN scale
    folded on host), BN-bias + ReLU, bf16 y write.

    Single input tensor: a meta header (iota, iota column, bias, folded conv
    weight, one-hot idx columns -- f32 values shipped as raw bf16 pairs)
    followed by the 128x80 value tiles, so the first DMA carries everything
    the first matmul chain needs."""
    S = sum(len(b) for b in bank_tiles)
    H_tot = sum(h for b in bank_tiles for _, h in b)
    T_u = sum(k + h for b in bank_tiles for k, h in b)
    MOFF = 260 + 2 * H_tot       # bf16 cols of meta header
    nc = bacc.Bacc("TRN2", target_bir_lowering=False, debug=False, num_devices=NCORES)
    vals = nc.dram_tensor("vals", [128, MOFF + T_u * CC], BF16, kind="ExternalInput")
    y = nc.dram_tensor("y", [OC, S * BW], BF16, kind="ExternalOutput")

    rest = T_u - b1
    bnd = [0, b1, b1 + rest // 3, b1 + (2 * rest) // 3, T_u]     # tile chunks

    with tile.TileContext(nc) as tc:
        with (
            tc.tile_pool(name="vals", bufs=4) as vals_pool,
            tc.tile_pool(name="ident", bufs=1) as ident_pool,
            tc.tile_pool(name="oh", bufs=max(1, H_tot)) as oh_pool,
            tc.tile_pool(name="bev", bufs=2) as bev_pool,
            tc.tile_pool(name="yst", bufs=1) as yst_pool,
            tc.tile_pool(name="psA", bufs=3, space="PSUM") as psA_pool,
            tc.tile_pool(name="psC", bufs=2, space="PSUM") as psC_pool,
        ):
            # prewarm the Activation function table at t=0
            warm = ident_pool.tile([128, 2], BF16)
            nc.gpsimd.memset(warm[:, 0:1], 0)
            nc.scalar.activation(
                out=warm[:, 1:2], in_=warm[:, 0:1],
                func=mybir.ActivationFunctionType.Relu,
            )
            vt = []
            for chk in range(4):
                lo = 0 if chk == 0 else MOFF + bnd[chk] * CC
                hi = MOFF + bnd[chk + 1] * CC
                t = vals_pool.tile([128, hi - lo], BF16)
                nc.sync.dma_start(out=t[:], in_=vals[:, lo:hi])
                vt.append(t)
            c0 = vt[0]
            iota_t = c0[:, 0:128]                           # (128,128) iota row
            iotac = c0[:, 128:130].bitcast(F32)             # (128,1) 0..127
            bias_ap = c0[:, 130:132].bitcast(F32)
            wS_t = c0[0:CC, 132:260]                        # (80,128) folded W
            idx_t = c0[:, 260:260 + 2 * H_tot].bitcast(F32) if H_tot else None

            def val_slice(tf):
                chk = next(i for i in range(4) if bnd[i] <= tf < bnd[i + 1])
                lo = (tf - bnd[chk]) * CC + (MOFF if chk == 0 else 0)
                return vt[chk][:, lo:lo + CC]

            # shared identity rhs for the identity scatter tiles
            ident = ident_pool.tile([128, 128], BF16)
            nc.vector.tensor_scalar(
                out=ident[:], in0=iota_t, scalar1=iotac, scalar2=None,
                op0=mybir.AluOpType.is_equal,
            )
            # build all one-hot tiles upfront (only need meta); 2:1 DVE:Pool
            ohs = []
            for j in range(H_tot):
                oh = oh_pool.tile([128, 128], BF16)
                oheng = nc.gpsimd if (j % 3 == 2) else nc.vector
                oheng.tensor_scalar(
                    out=oh[:], in0=iota_t, scalar1=idx_t[:, j:j + 1],
                    scalar2=None, op0=mybir.AluOpType.is_equal,
                )
                ohs.append(oh)

            yst = yst_pool.tile([OC, S * BW], BF16)
            copy_eng = (nc.vector, nc.scalar, nc.vector, nc.vector)
            relu_eng = (nc.scalar, nc.vector, nc.scalar, nc.vector)
            ydma_eng = (nc.sync, nc.gpsimd, nc.gpsimd, nc.sync)
            tf = 0
            oh_i = 0
            ybase = 0
            for q, bank in enumerate(bank_tiles):
                nsl = len(bank)
                bcols = nsl * BW
                ps = psA_pool.tile([128, bcols], F32, space="PSUM")
                for i, (nid, noh) in enumerate(bank):
                    nchain = nid + noh
                    for k in range(nchain):
                        nc.tensor.matmul(
                            out=ps[0:CC, i * BW:(i + 1) * BW],
                            lhsT=val_slice(tf),
                            rhs=ident[:] if k < nid else ohs[oh_i][:],
                            start=(k == 0),
                            stop=(k == nchain - 1),
                        )
                        if k >= nid:
                            oh_i += 1
                        tf += 1
                # drain bank: copy -> conv -> BN bias + ReLU; stages spread
                # over DVE/Act so consecutive bank drains pipeline cleanly
                bev = bev_pool.tile([CC, bcols], BF16)
                ce = copy_eng[q % 4]
                (ce.tensor_copy if ce is nc.vector else ce.copy)(
                    out=bev[:], in_=ps[0:CC, :])
                psc = psC_pool.tile([OC, bcols], F32, space="PSUM")
                nc.tensor.matmul(
                    out=psc[:], lhsT=wS_t, rhs=bev[:], start=True, stop=True
                )
                yr = yst[:, ybase:ybase + bcols]
                re = relu_eng[q % 4]
                if re is nc.scalar:
                    re.activation(
                        out=yr, in_=psc[:],
                        func=mybir.ActivationFunctionType.Relu,
                        bias=bias_ap, scale=1.0,
                    )
                else:
                    re.tensor_scalar(
                        out=yr, in0=psc[:], scalar1=bias_ap, scalar2=0.0,
                        op0=mybir.AluOpType.add, op1=mybir.AluOpType.max,
                    )
                ydma_eng[q % 4].dma_start(
                    out=y[:, ybase:ybase + bcols], in_=yr)
                ybase += bcols
    nc.compile()
    return nc


# ---------------------------------------------------------------- host plan
def _compute_bins(intrinsics, cam2ego):
    """Mirror the reference's index math exactly (same jnp ops, same backend)
    so floor() results match bit-for-bit, then reduce over the h axis."""
    import jax.numpy as jnp

    intrinsics = jnp.asarray(intrinsics)
    cam2ego = jnp.asarray(cam2ego)
    u = ((jnp.arange(WF, dtype=jnp.float32) + 0.5) * STRIDE)[None, None, None, None, :]
    v = ((jnp.arange(HF, dtype=jnp.float32) + 0.5) * STRIDE)[None, None, None, :, None]
    Z = jnp.linspace(Z_MIN, Z_MAX, DD, dtype=jnp.float32)[None, None, :, None, None]

    fx = intrinsics[:, :, 0, 0][:, :, None, None, None]
    fy = intrinsics[:, :, 1, 1][:, :, None, None, None]
    cx = intrinsics[:, :, 0, 2][:, :, None, None, None]
    cy = intrinsics[:, :, 1, 2][:, :, None, None, None]

    Xc = (u - cx) / fx * Z
    Yc = (v - cy) / fy * Z
    Zc = jnp.broadcast_to(Z, Xc.shape)

    T = cam2ego[:, :, None, None, None]
    x_e = T[..., 0, 0] * Xc + T[..., 0, 1] * Yc + T[..., 0, 2] * Zc + T[..., 0, 3]
    y_e = T[..., 1, 0] * Xc + T[..., 1, 1] * Yc + T[..., 1, 2] * Zc + T[..., 1, 3]

    mx = (PC[3] - PC[0]) / BW
    my = (PC[4] - PC[1]) / BH
    ix = jnp.floor((x_e - PC[0]) / mx).astype(jnp.int32)
    iy = jnp.floor((y_e - PC[1]) / my).astype(jnp.int32)
    valid = (ix >= 0) & (ix < BW) & (iy >= 0) & (iy < BH)

    ix = np.asarray(ix)[0]
    iy = np.asarray(iy)[0]
    valid = np.asarray(valid)[0]
    # h-independence (holds for z-yaw-only rigs; required by this kernel)
    assert (ix == ix[:, :, :1, :]).all() and (iy == iy[:, :, :1, :]).all() and (
        valid == valid[:, :, :1, :]
    ).all(), "BEV bin depends on image row; kernel assumes z-yaw-only rig"
    return ix[:, :, 0, :], iy[:, :, 0, :], valid[:, :, 0, :]   # (N, D, W)


def _plan(intrinsics, cam2ego):
    ix, iy, valid = _compute_bins(intrinsics, cam2ego)
    # global point id = ray*DD + d, ray = n*WF + w
    ixr = ix.transpose(0, 2, 1).reshape(-1).astype(np.int64)   # (n, w, d) flat
    iyr = iy.transpose(0, 2, 1).reshape(-1).astype(np.int64)
    vr = valid.transpose(0, 2, 1).reshape(-1)
    pid = np.arange(RAYS * DD, dtype=np.int64)

    # L1 ray geometry: per-ray contiguous valid-d windows, rays sorted by
    # window length desc within each core, uniform window dlen = max over
    # cores per position (SPMD: one program for all cores)
    vrw = vr.reshape(RAYS, DD)
    lens = vrw.sum(1).astype(np.int64)
    assert (lens > 0).all()
    dlo = np.argmax(vrw, axis=1).astype(np.int64)
    ray_order = []
    for c in range(NCORES):
        ids = np.arange(c * RPC, (c + 1) * RPC)
        ray_order.append(ids[np.argsort(-lens[ids], kind="stable")])
    dlen = tuple(int(x) for x in np.max([lens[o] for o in ray_order], axis=0))

    vpid = pid[vr]
    vbin = (iyr * BW + ixr)[vr]
    cnt = np.bincount(vbin, minlength=BH * BW)

    nz = np.where(cnt > 0)[0]
    order = nz[np.argsort(-cnt[nz], kind="stable")]
    # snake-deal nonzero bins to cores by descending count -> balanced points
    core_bins = [[] for _ in range(NCORES)]
    for i, b in enumerate(order):
        rnd, pos = divmod(i, NCORES)
        c = pos if rnd % 2 == 0 else NCORES - 1 - pos
        core_bins[c].append(int(b))
    S = max(1, math.ceil(max(len(x) for x in core_bins) / 128))

    # per-bin point lists (pids), grouped
    binpts = {}
    o2 = np.argsort(vbin, kind="stable")
    sb = vbin[o2]
    sp = vpid[o2]
    starts = np.searchsorted(sb, nz)
    ends = np.searchsorted(sb, nz, side="right")
    for b, lo, hi in zip(nz, starts, ends):
        binpts[int(b)] = sp[lo:hi]

    # K0[s]: identity passes while on average >= half the slot is active.
    # H[s]: one-hot tiles for the overflow (uniform = max over cores).
    K0, H = [], []
    for s in range(S):
        cnts = [np.array([cnt[b] for b in core_bins[c][s * 128:(s + 1) * 128]]
                         or [0]) for c in range(NCORES)]
        k0 = 1
        while np.mean([(cc > k0).sum() for cc in cnts]) >= 64:
            k0 += 1
        kmax = max(int(cc.max()) for cc in cnts)
        k0 = min(k0, max(1, kmax))
        ov = max(int(np.maximum(0, cc - k0).sum()) for cc in cnts)
        K0.append(k0)
        H.append(math.ceil(ov / 128))

    # deal slots to PSUM banks, balancing tiles per bank.  S = 4*(NB-1)+r:
    # the last bank gets the r slots whose tile counts are closest to the
    # per-bank average; the rest go greedily (heaviest to lightest bank).
    tiles = [K0[s] + H[s] for s in range(S)]
    NB = math.ceil(S / 4)
    r = S - 4 * (NB - 1)
    avg = sum(tiles) / NB
    by_sz = sorted(range(S), key=lambda s: -tiles[s])
    lastb = sorted(range(S), key=lambda s: abs(4 * tiles[s] - avg))[:r]
    banks = [[] for _ in range(NB - 1)]
    loads = [0] * (NB - 1)
    for s in by_sz:
        if s in lastb:
            continue
        free = [q for q in range(NB - 1) if len(banks[q]) < 4]
        i = min(free, key=lambda q: (loads[q], len(banks[q])))
        banks[i].append(s)
        loads[i] += tiles[s]
    banks.append(sorted(lastb))
    bank_tiles = tuple(tuple((K0[s], H[s]) for s in b) for b in banks)
    slot_order = [s for b in banks for s in b]    # emission order of slots
    return dict(
        K0=tuple(K0), H=tuple(H), S=S, bank_tiles=bank_tiles,
        slot_order=slot_order,
        core_bins=core_bins, binpts=binpts, cnt=cnt,
        dlo=dlo, lens=lens, ray_order=ray_order, dlen=dlen,
    )


# ---------------------------------------------------------------- main entry
def _l1_inputs(depth_prob, context):
    dt = _np_bf16()
    dT = depth_prob[0].transpose(2, 0, 3, 1).reshape(HF, RAYS, DD)  # h, ray, d
    cT = context[0].transpose(2, 0, 3, 1).reshape(HF, RAYS, CC)     # h, ray, c
    maps = []
    for c in range(NCORES):
        sl = slice(c * RPC, (c + 1) * RPC)
        d4 = dT[:, sl].reshape(HF, GPC, 2, 2, DD)    # h, g, half, pj, d
        c4 = cT[:, sl].reshape(HF, GPC, 2, 2, CC)
        dc = np.concatenate([d4, c4], axis=-1)       # h, g, half, pj, 128
        dc = dc.transpose(2, 0, 1, 3, 4).reshape(128, GPC * 2 * PAIR)
        maps.append({"dc_in": np.ascontiguousarray(dc).astype(dt)})
    return maps


def _l2_inputs(plan, part_all, w_proj, b_proj, bn_gamma, bn_beta, bn_mean, bn_var):
    dt = _np_bf16()
    K0, H = plan["K0"], plan["H"]
    H_tot = sum(H)
    T_u = sum(K0) + H_tot
    cnt, binpts = plan["cnt"], plan["binpts"]
    scale = (bn_gamma / np.sqrt(bn_var + BN_EPS)).astype(np.float32)
    bias = ((b_proj - bn_mean) * scale + bn_beta).astype(np.float32)
    # fold BN scale into the conv weight; device conv runs after the scatter.
    wS = (w_proj * scale[:, None]).astype(np.float32)        # (OC, CC)
    # meta header in bf16 columns: 0:128 iota row; 128:130 iota column (f32
    # raw); 130:132 bias (f32 raw); 132:260 folded weight; 260:260+2H idx
    iota_row = np.broadcast_to(
        np.arange(128, dtype=np.float32).astype(dt), (128, 128))
    iotac_raw = np.arange(128, dtype=np.float32)[:, None].astype(
        np.float32).view(np.uint32).view(np.uint16).reshape(128, 2)
    bias_raw = bias[:, None].view(np.uint32).view(np.uint16).reshape(128, 2)
    wS_bf = np.zeros((128, 128), dt)
    wS_bf[0:CC] = np.ascontiguousarray(wS.T.astype(dt))

    maps = []
    for c in range(NCORES):
        bins = plan["core_bins"][c]
        vals = np.zeros((128, T_u, CC), np.float32)
        idxs = np.full((128, max(1, H_tot)), -1.0, np.float32)
        tf = 0
        oh_i = 0
        for s in plan["slot_order"]:
            sl_bins = bins[s * 128:(s + 1) * 128]
            for k in range(K0[s]):
                for col, b in enumerate(sl_bins):
                    if cnt[b] > k:
                        vals[col, tf] = part_all[binpts[b][k]]
                tf += 1
            ov = [(col, p) for col, b in enumerate(sl_bins)
                  for p in binpts[b][K0[s]:]]
            for j in range(H[s]):
                seg = ov[j * 128:(j + 1) * 128]
                for r, (col, p) in enumerate(seg):
                    vals[r, tf] = part_all[p]
                    idxs[r, oh_i] = col
                tf += 1
                oh_i += 1
        idx_raw = np.ascontiguousarray(
            idxs[:, :H_tot]).view(np.uint32).view(np.uint16).reshape(
                128, 2 * H_tot)
        header = np.concatenate(
            [np.asarray(iota_row).view(np.uint16),
             iotac_raw, bias_raw,
             wS_bf.view(np.uint16), idx_raw], axis=1)
        full = np.concatenate(
            [header.view(dt), vals.reshape(128, -1).astype(dt)], axis=1)
        maps.append({"vals": np.ascontiguousarray(full)})
    return maps, bias


def kernel(**inputs) -> np.ndarray:
    depth_prob = np.asarray(inputs["depth_prob"], np.float32)
    context = np.asarray(inputs["context"], np.float32)
    intrinsics = np.asarray(inputs["intrinsics"], np.float32)
    cam2ego = np.asarray(inputs["cam2ego"], np.float32)

    plan = _plan(intrinsics, cam2ego)
    nc1 = _build_l1f(plan["dlen"])
    l1_maps = _l1f_inputs(depth_prob, context, plan)
    res1 = run_bass_kernel_spmd(nc1, l1_maps, list(range(NCORES))).results

    part_all = _unpack_l1f([res1[c]["part"] for c in range(NCORES)], plan)

    nc2 = _build_l2(plan["bank_tiles"], 3)
    l2_maps, bias = _l2_inputs(
        plan,
        part_all,
        np.asarray(inputs["w_proj"], np.float32),
        np.asarray(inputs["b_proj"], np.float32),
        np.asarray(inputs["bn_gamma"], np.float32),
        np.asarray(inputs["bn_beta"], np.float32),
        np.asarray(inputs["bn_mean"], np.float32),
        np.asarray(inputs["bn_var"], np.float32),
    )
    res2 = run_bass_kernel_spmd(nc2, l2_maps, list(range(NCORES))).results

    # dead (zero-count) bins: y = ReLU(bias), a weight-derived constant
    y = np.empty((1, OC, BH, BW), np.float32)
    y[0] = np.maximum(bias, 0.0)[:, None, None]
    for c in range(NCORES):
        yc = np.asarray(res2[c]["y"], np.float32)      # (OC, S*BW)
        bins = plan["core_bins"][c]
        yf = y[0].reshape(OC, BH * BW)
        for pos, s in enumerate(plan["slot_order"]):
            sl = bins[s * 128:(s + 1) * 128]
            yf[:, sl] = yc[:, pos * 128:pos * 128 + len(sl)]
    return y
